# revision 20
# baseline (speedup 1.0000x reference)
"""Causal self-attention on 8 TRN2 NeuronCores.

Problem (hardcoded): B=2, L=2048, D=1024, H=16 heads, Hd=64, fp32 in/out.
    q = x@Wq.T+bq ... scores = q k^T/sqrt(Hd), causal softmax, out = (attn v)@Wo.T+bo

Sharding:
  - Tensor-parallel over heads: core c owns heads 2c, 2c+1 (128 cols of q/k/v).
    QKV projections + attention are fully local per core.
  - Output projection is token-parallel: core c owns tokens
    {Q*1024 + c*128 + j : Q in 0..3, j in 0..127}.  The attention outputs are
    re-sharded head-major -> token-major with FOUR 256KB AllToAlls, each
    issued as soon as its quarter of the tokens finishes attention, so the
    collectives and the o-proj overlap the remaining attention compute.

Vs the fp32r baseline:
  - bf16 on-chip (PSUM accum fp32; output fp32).  Same matmul rate as fp32r
    but DMA/SBUF/DVE costs halve.  rel err ~1e-3 (budget 2e-2).
  - bk dropped entirely (softmax is invariant to per-row score shifts).
  - bv folded into bo on the host (softmax weights sum to 1, so bv passes
    through attention additively: bo_eff = bo + bv@Wo.T).
  - v computed directly in natural [token, vd] layout by using the xT tile as
    the stationary operand (no PE transposes, no DVE repack).
  - softmax normalization: denominator from a ones-column in v (PV row 64);
    1/denom broadcast across partitions with a small ones-matmul into PSUM
    (no DRAM bounce), then one DVE multiply per head.
  - attention is ACT(exp)-paced: the per-chunk projection and o-proj work is
    sliced into small units and woven between attention score groups so PE
    always has queued work while ACT leads.
  - diagonal (masked) group first + PV one group behind the score matmuls.
  - DMAs batched and issued from two queues by dependency class: pure loads
    on SP, producer-dependent stores on gpsimd/SWDGE.
"""

import math
import sys

import numpy as np

sys.path.insert(0, "/opt/trn_rl_repo")

import ml_dtypes

import concourse.bass as bass
import concourse.bacc as bacc
import concourse.mybir as mybir
import concourse.tile as tile
from concourse.bass_utils import run_bass_kernel_spmd

F32 = mybir.dt.float32
F32R = mybir.dt.float32r
BF16 = mybir.dt.bfloat16
AF = mybir.ActivationFunctionType
ALU = mybir.AluOpType

NC = 8          # cores
B, L, D = 2, 2048, 1024
H, HD = 16, 64
T = B * L       # 4096
TPC = T // NC   # 512 tokens per core for o-proj
SCALE = 1.0 / math.sqrt(HD)  # 0.125
NQ = 4          # a2a quarters (1024 tokens each)


def build_bass(a2a=True):
    nc = bacc.Bacc("TRN2", target_bir_lowering=False, debug=False, num_devices=NC)

    xT = nc.declare_dram_parameter("xT", [D, T], BF16, isOutput=False)
    # host pre-shuffles each to [128 p, 8 kc, 128 m] contiguous
    wq = nc.declare_dram_parameter("wq", [128, 8 * 128], BF16, isOutput=False)
    wk = nc.declare_dram_parameter("wk", [128, 8 * 128], BF16, isOutput=False)
    wv = nc.declare_dram_parameter("wv", [128, 8 * 128], BF16, isOutput=False)
    bq = nc.declare_dram_parameter("bq", [128, 1], F32, isOutput=False)
    # host pre-shuffles Wo.T: [128 p, 8 s, 1024 oc]
    wo = nc.declare_dram_parameter("wo", [128, 8 * 1024], BF16, isOutput=False)
    bo = nc.declare_dram_parameter("bo", [128, 8], F32, isOutput=False)  # bo_eff
    mask = nc.declare_dram_parameter("mask", [128, 1024], BF16, isOutput=False)
    out_e = nc.declare_dram_parameter("out", [8, 128, TPC], F32, isOutput=True)

    # internal DRAM for the quarter all-to-alls: [dest core, hd dim, token]
    a2a_in = [nc.dram_tensor(f"a2a_in{q}", [NC, 128, 128], BF16) for q in range(NQ)]
    a2a_out = [nc.dram_tensor(f"a2a_out{q}", [NC, 128, 128], BF16) for q in range(NQ)]

    with tile.TileContext(nc) as tc:
        with (
            tc.tile_pool(name="persist", bufs=1) as persist,
            tc.tile_pool(name="xt", bufs=3) as xt_pool,
            # PSUM budget (8 banks): sc 2x[128,1024]=4, pv0/pv1 1 bank each,
            # work 2 (shared by stage-A proj, v blocks, 1/sum bcast, o-proj)
            tc.tile_pool(name="work", bufs=2, space="PSUM") as work_pool,
            tc.tile_pool(name="sc", bufs=2, space="PSUM") as sc_pool,
            tc.tile_pool(name="pv", bufs=1, space="PSUM") as pv_pool,
            tc.tile_pool(name="es", bufs=3) as es_pool,
            tc.tile_pool(name="sm", bufs=4) as sm_pool,
            tc.tile_pool(name="ob", bufs=2) as ob_pool,
        ):
            # ---- q weights + first x chunk first: PE can start ~2.5us in ----
            wq_sb = persist.tile([128, 8, 128], BF16, tag="wq")
            nc.sync.dma_start(out=wq_sb[:, :, :], in_=wq[:, :])

            def load_xt(tb):  # one 512-token chunk, 2 DMAs of 4 kc-tiles
                ts0 = tb * 512
                xt_t = xt_pool.tile([128, 8, 512], BF16, tag="xt")
                base = xT[0:128, 0:512]  # AP template for tensor/offset
                for half in range(2):
                    src = bass.AP(
                        tensor=base.tensor,
                        offset=base.offset + half * 4 * 128 * T + ts0,
                        ap=[[T, 128], [128 * T, 4], [1, 512]],
                    )
                    nc.sync.dma_start(out=xt_t[:, half * 4:(half + 1) * 4, :],
                                      in_=src)
                return xt_t

            xt0 = load_xt(0)

            wk_sb = persist.tile([128, 8, 128], BF16, tag="wk")
            nc.sync.dma_start(out=wk_sb[:, :, :], in_=wk[:, :])
            wv_sb = persist.tile([128, 8, 128], BF16, tag="wv")
            nc.sync.dma_start(out=wv_sb[:, :, :], in_=wv[:, :])

            # ---- remaining constants / weights ----
            bq_sb = persist.tile([128, 1], F32, tag="bq")
            nc.sync.dma_start(out=bq_sb[:, :], in_=bq[:, :])
            mask_sb = persist.tile([128, 1024], BF16, tag="mask")
            nc.sync.dma_start(out=mask_sb[:, :], in_=mask[:, :])
            bo_sb = persist.tile([128, 8], F32, tag="bo")
            nc.sync.dma_start(out=bo_sb[:, :], in_=bo[:, :])
            ones_sb = persist.tile([1, 64], F32R, tag="ones")
            nc.vector.memset(ones_sb[:, :].bitcast(F32), 1.0)
            wo_sb = persist.tile([128, 8, 1024], BF16, tag="wo")

            # ---- resident activations ----
            qT_sb = persist.tile([128, T], BF16, tag="qT")
            kT_sb = persist.tile([128, T], BF16, tag="kT")
            # v natural layout: 32 token blocks x [128 t, (v_h0|1|v_h1|1)]
            v_sb = persist.tile([128, 32, 130], BF16, tag="v")
            nc.vector.memset(v_sb[:, :, 64:65], 1.0)
            nc.vector.memset(v_sb[:, :, 129:130], 1.0)
            g_sb = persist.tile([128, 8, TPC], BF16, tag="gath")

            # ============ stage A: QKV projections, as 6 units ============
            def stage_a_units(tb, xt_t):  # one 512-token chunk
                ts_ = slice(tb * 512, (tb + 1) * 512)

                def qk_unit(w_sb, b_sb, dst):
                    def f():
                        ps = work_pool.tile([128, 512], F32, tag="work")
                        for kc in range(8):
                            nc.tensor.matmul(
                                ps[:, :], w_sb[:, kc, :], xt_t[:, kc, :],
                                start=(kc == 0), stop=(kc == 7),
                            )
                        if b_sb is not None:
                            nc.scalar.activation(
                                dst[:, ts_], ps[:, :], AF.Identity,
                                bias=b_sb[:, :], scale=1.0,
                            )
                        else:
                            nc.vector.tensor_copy(dst[:, ts_], ps[:, :])
                    return f

                def v_unit(s):
                    def f():
                        # v in natural [token, vd] layout: xT is stationary
                        blk = tb * 4 + s
                        vp = work_pool.tile([128, 128], F32, tag="work")
                        for kc in range(8):
                            nc.tensor.matmul(
                                vp[:, :], xt_t[:, kc, s * 128:(s + 1) * 128],
                                wv_sb[:, kc, :],
                                start=(kc == 0), stop=(kc == 7),
                            )
                        dstv = bass.AP(
                            tensor=v_sb.tensor, offset=v_sb.offset + blk * 130,
                            ap=[list(v_sb.ap[0]), [65, 2], [1, 64]],
                        )
                        srcv = bass.AP(
                            tensor=vp.tensor, offset=vp.offset,
                            ap=[list(vp.ap[0]), [64, 2], [1, 64]],
                        )
                        nc.vector.tensor_copy(dstv, srcv)
                    return f

                return [
                    qk_unit(wq_sb, bq_sb, qT_sb),
                    qk_unit(wk_sb, None, kT_sb),
                    v_unit(0), v_unit(1), v_unit(2), v_unit(3),
                ]

            # ================= stage B: attention =================
            def att(bb, qc, feed, drain=False):  # one 256-query block
                t0 = bb * 2048
                qs = slice(t0 + qc * 256, t0 + qc * 256 + 256)
                # one PSUM bank per head: a start=True clears has_written
                # for its whole bank, so the heads must not share one.
                pvs = [
                    pv_pool.tile([65, 256], F32, tag=f"pv{h}", name=f"pv{h}")
                    for h in range(2)
                ]
                # diagonal (masked) group first, and PV one group behind the
                # score matmuls, so the exp->mask chain of group gi overlaps
                # the score matmuls of group gi+1 instead of stalling PE.
                order = [qc] + list(range(qc))
                ess = []

                def emit_pv(gi):
                    g, es = ess[gi]
                    for h in range(2):
                        for jj in range(2):
                            blk = bb * 16 + 2 * g + jj
                            nc.tensor.matmul(
                                pvs[h][:, :],
                                v_sb[:, blk, h * 65:h * 65 + 65],
                                es[:, h * 512 + jj * 256: h * 512 + jj * 256 + 256],
                                start=(gi == 0 and jj == 0),
                                stop=(gi == len(order) - 1 and jj == 1),
                            )

                for gi, g in enumerate(order):
                    if gi % 3 == 1:  # weave one background unit in
                        u = next(feed, None)
                        if u is not None:
                            u()
                    sc = sc_pool.tile([128, 1024], F32, tag="sc")
                    es = es_pool.tile([128, 1024], BF16, tag="es")
                    for h in range(2):
                        hs = slice(h * 64, (h + 1) * 64)
                        for jj in range(2):
                            j = 2 * g + jj
                            ks = slice(t0 + j * 128, t0 + j * 128 + 128)
                            nc.tensor.matmul(
                                sc[:, h * 512 + jj * 256: h * 512 + jj * 256 + 256],
                                kT_sb[hs, ks], qT_sb[hs, qs],
                                start=True, stop=True,
                            )
                    nc.scalar.activation(es[:, :], sc[:, :], AF.Exp, scale=SCALE)
                    if g == qc:  # diagonal pair of key blocks -> causal mask
                        nc.vector.tensor_tensor(
                            out=es[:, :], in0=es[:, :], in1=mask_sb[:, :],
                            op=ALU.mult,
                        )
                    ess.append((g, es))
                    if gi >= 1:
                        emit_pv(gi - 1)
                emit_pv(len(order) - 1)
                if drain:
                    # emit leftover background units before the normalize
                    # chain so their PSUM tiles don't queue behind it
                    for u in feed:
                        u()
                # normalize per head: 1/sum broadcast across partitions via a
                # small ones-matmul into PSUM, then one DVE multiply.
                tq = bb * 2048 + qc * 256
                Q = tq // 1024
                d0 = ((qc % 4) * 256) // 128
                a2aQ = a2a_in[Q][:, :, :]
                for h in range(2):
                    rec = sm_pool.tile([1, 256], F32R, tag="rec", bufs=4,
                                       name="rec")
                    with nc.allow_low_precision(reason="f32r 1/sum is exact enough"):
                        nc.vector.reciprocal(rec[:, :], pvs[h][64:65, :])
                    unn = sm_pool.tile([64, 256], F32, tag="unn", bufs=4,
                                       name="unn")
                    nc.vector.tensor_copy(unn[:, :], pvs[h][0:64, :])
                    bc = work_pool.tile([64, 256], F32, tag="work")
                    nc.tensor.matmul(
                        bc[:, :], ones_sb[:, :], rec[:, :],
                        start=True, stop=True,
                    )
                    att_t = sm_pool.tile([64, 256], BF16, tag="att", bufs=4,
                                         name="att")
                    nc.vector.tensor_tensor(
                        out=att_t[:, :], in0=unn[:, :],
                        in1=bc[:, :], op=ALU.mult,
                    )
                    # att_t [64 hd, 256 t] -> a2a_in_Q[d0:d0+2, h*64:+64, :]
                    dst = bass.AP(
                        tensor=a2aQ.tensor,
                        offset=a2aQ.offset + d0 * 16384 + h * 64 * 128,
                        ap=[[128, 64], [16384, 2], [1, 128]],
                    )
                    src = bass.AP(
                        tensor=att_t.tensor, offset=att_t.offset,
                        ap=[list(att_t.ap[0]), [128, 2], [1, 128]],
                    )
                    nc.gpsimd.dma_start(out=dst, in_=src)

            # ================= quarter all-to-all + gather =================
            def a2a_q(Q):
                if a2a:
                    nc.gpsimd.collective_compute(
                        "AllToAll", ALU.bypass,
                        replica_groups=[list(range(NC))],
                        ins=[a2a_in[Q][:, :, :].opt()],
                        outs=[a2a_out[Q][:, :, :].opt()],
                    )
                else:  # single-core timing variant: stand-in local copy
                    nc.gpsimd.dma_start(out=a2a_out[Q][:, :, :].opt(),
                                        in_=a2a_in[Q][:, :, :].opt())

            def gather_q(Q):
                # gather [8 src, 128 hd, 128 t] -> g_sb[:, src, Q*128:+128]
                aoQ = a2a_out[Q][:, :, :]
                src = bass.AP(
                    tensor=aoQ.tensor, offset=aoQ.offset,
                    ap=[[128, 128], [16384, 8], [1, 128]],
                )
                dstg = bass.AP(
                    tensor=g_sb.tensor, offset=g_sb.offset + Q * 128,
                    ap=[list(g_sb.ap[0]), [TPC, 8], [1, 128]],
                )
                nc.gpsimd.dma_start(out=dstg, in_=src)

            # ========== stage C: output projection, as 1+8 units ==========
            def oproj_units(Q):
                osb = ob_pool.tile([128, 8, 128], F32, tag="ob")

                def ob_unit(ob):
                    def f():
                        pso = work_pool.tile([128, 128], F32, tag="work")
                        for s in range(8):
                            nc.tensor.matmul(
                                pso[:, :],
                                wo_sb[:, s, ob * 128:(ob + 1) * 128],
                                g_sb[:, s, Q * 128:(Q + 1) * 128],
                                start=(s == 0), stop=(s == 7),
                            )
                        # drain on DVE (ACT is busy with exp): add per-
                        # partition bias while copying PSUM -> SBUF
                        nc.vector.tensor_scalar_add(
                            osb[:, ob, :], pso[:, :], bo_sb[:, ob:ob + 1],
                        )
                        if ob == 7:
                            # osb [128 p, 8 ob, 128 j] -> out_e[ob, p, Q*128+j]
                            oe = out_e[:, :, :]
                            dsto = bass.AP(
                                tensor=oe.tensor, offset=oe.offset + Q * 128,
                                ap=[[TPC, 128], [128 * TPC, 8], [1, 128]],
                            )
                            nc.gpsimd.dma_start(out=dsto, in_=osb[:, :, :])
                    return f

                return [lambda: gather_q(Q)] + [ob_unit(ob) for ob in range(8)]

            # ================= schedule =================
            # att pair k covers chunk k's queries; stage A for chunk k+1 and
            # ready o-proj quarters are fed into pair k's group loop.
            for u in stage_a_units(0, xt0):
                u()
            for k in range(8):
                bbk, qp = k // 4, k % 4
                feed_units = []
                if k < 7:
                    xt_t = load_xt(k + 1)
                    feed_units += stage_a_units(k + 1, xt_t)
                if k == 0:
                    nc.sync.dma_start(out=wo_sb[:, :, :], in_=wo[:, :])
                if k == 2:
                    feed_units += oproj_units(0)
                if k == 5:
                    feed_units += oproj_units(1)
                if k == 7:
                    feed_units += oproj_units(2)
                feed = iter(feed_units)
                att(bbk, 2 * qp, feed)
                att(bbk, 2 * qp + 1, feed, drain=True)
                if k == 1:
                    a2a_q(0)
                if k == 3:
                    a2a_q(1)
                if k == 5:
                    a2a_q(2)
                if k == 7:
                    a2a_q(3)
            for u in oproj_units(3):
                u()

    nc.compile()
    return nc


_BUILT = None


def _get_built():
    global _BUILT
    if _BUILT is None:
        _BUILT = build_bass()
    return _BUILT


def _shuf_w(W, sl):
    """W[sl,:].T as [128 p, 8 kc, 128 m] -> [128, 1024] bf16 contiguous."""
    bf = ml_dtypes.bfloat16
    w = W[sl, :].T.astype(bf).reshape(8, 128, 128).transpose(1, 0, 2)
    return np.ascontiguousarray(w.reshape(128, 1024))


def _make_in_maps(x, Wq, bq, Wk, bk, Wv, bv, Wo, bo):
    bf = ml_dtypes.bfloat16
    xT = np.ascontiguousarray(x.reshape(T, D).T.astype(bf))        # [D, T]
    bo_eff = bo + bv @ Wo.T                                        # fold bv
    bo_r = np.ascontiguousarray(bo_eff.reshape(8, 128).T.astype(np.float32))
    # Wo.T pre-shuffled: [128 p, 8 s, 1024 oc] -> [128, 8192]
    woT = Wo.T.astype(bf).reshape(8, 128, 1024).transpose(1, 0, 2)
    woT = np.ascontiguousarray(woT.reshape(128, 8192))

    cc = np.arange(256)[None, :]
    rr = np.arange(128)[:, None]
    maskA = (rr <= cc).astype(bf)
    maskB = ((128 + rr) <= cc).astype(bf)
    mask = np.concatenate([maskA, maskB, maskA, maskB], axis=1)  # [128, 1024]

    in_maps = []
    for c in range(NC):
        sl = slice(c * 128, (c + 1) * 128)
        in_maps.append({
            "xT": xT,
            "wq": _shuf_w(Wq, sl),
            "wk": _shuf_w(Wk, sl),
            "wv": _shuf_w(Wv, sl),
            "bq": np.ascontiguousarray(bq[sl][:, None].astype(np.float32)),
            "wo": woT,
            "bo": bo_r,
            "mask": mask,
        })
    return in_maps


def run(inputs, trace=False):
    """Run on hardware; returns (output [B, L, D], BassKernelResults)."""
    inputs = {k: np.asarray(v, dtype=np.float32) for k, v in inputs.items()}
    nc = _get_built()
    in_maps = _make_in_maps(**inputs)
    res = run_bass_kernel_spmd(
        nc, in_maps, core_ids=list(range(NC)), trace=trace,
    )
    # out[c] = [8, 128, 512]: free idx = Q*128 + j -> token Q*1024 + c*128 + j
    outT = np.empty((D, T), np.float32)
    for c in range(NC):
        o = res.results[c]["out"]  # [8, 128, TPC]
        for Q in range(NQ):
            outT[:, Q * 1024 + c * 128: Q * 1024 + (c + 1) * 128] = (
                o[:, :, Q * 128:(Q + 1) * 128].reshape(D, 128)
            )
    out = np.ascontiguousarray(outT.T).reshape(B, L, D)
    return out, res


def kernel(**inputs):
    out, _ = run(inputs, trace=False)
    return out
